# revision 9
# baseline (speedup 1.0000x reference)
"""Trainium2 Bass kernel v3 for nn_EnhancedKeypointLoss.

Key techniques:
  - 2 ScalarE table loads total (all Sqrts, then Arctan/Sigmoid set; the
    base-loss exp is computed via exp(-e) = 1/sigmoid(e) - 1).
  - Signed half-angle phi = 2*atan(dy/(n+dx)); HW arctan table is accurate
    over the full fp32 range, so no quadrant fixup.
  - bf16 throughout the bulk domains; 8 row-tiles batched per instruction.
  - Custom DVE ops (registered at import):
      ANT_SQ_SUM:   out = Src0^2 + Src1^2            (nsq in one pass)
      ANT_RECIP_MUL:out = Src1 * approx(1/Src0)      (seed + 1 NR step)
      ANT_WRAP_DIFF:out = |Src0 - Src1| + C0         (pair delta + wrap)
      ANT_ABSDIFF_SQ_ACC: out=(|Src0|-|Src1|)^2, accum=sum
  - Wrapped-angle big domain [8 tiles x 8 shifts x 17 x 18] with the phi
    table transposed and j-padded to 18 (pad column zeroed -> terms cancel).
  - V/S engine balance: a subset of the 16 (x, d) wrap slices is routed
    through ScalarE (two Abs passes) instead of the VectorE custom op.
"""

import numpy as np
from operator import add as _op_add

N_CORES = 8
N = 8192
K = 17
NLOC = N // N_CORES  # 1024
NT = NLOC // 128  # 8 tiles per core
KK = K * K  # 289
TK = NT * KK  # 2312
J = 17  # j dimension (no padding: custom DVE ops run 1x, no alignment constraint)
I25 = 25  # i rows incl. 8 wrap rows
TBL_T = I25 * J  # 450 per tile
TBL = NT * TBL_T  # 3600
DSLICE = NT * K * J  # 2448 elements per (x, d) wrap slice
PI = float(np.float32(np.pi))
W_BASE, W_RATIO, W_ANGLE = 1.0, 0.2, 0.2

# (x, d) slices routed through the ScalarE abs-chain instead of the V custom
S_PATH = {('g', 1), ('g', 3), ('g', 5), ('g', 7)}

_CACHE = {}
PARTS = {'front': True, 'atan': True, 'wrap': True, 'acc': True, 'plane': True, 'ratio': True, 'base': True}


def _register_custom_ops():
    if "ops" in _CACHE:
        return _CACHE["ops"]
    import concourse.dve_ops as dve_ops
    from concourse.dve_spec import Spec, Src0, Src1, C0, C1, C2, Bin, AluOp, maxx, sq, lower, Zero
    from concourse.dve_uop import DveOpSpec

    _xc = maxx(Src0, C2)
    _notx = Bin(AluOp.BITWISE_NOT, _xc, _xc)
    _y0 = _notx * C0
    specs = {
        "ANT_SQ_SUM": Spec(
            body=sq(Src0) + sq(Src1),
            reference=lambda in0, in1, s0, s1, imm2: (
                in0.astype(np.float32) ** 2 + in1.astype(np.float32) ** 2
            ),
        ),
        "ANT_RECIP_MUL": Spec(
            body=(_y0 * (C1 - _xc * _y0)) * Src1,
            reference=lambda in0, in1, s0, s1, imm2: (
                (lambda xc: (lambda y0: (y0 * (s1 - xc * y0)) * in1)(
                    (~xc.view(np.int32)).view(np.float32) * s0
                ))(np.maximum(in0.astype(np.float32), imm2))
            ),
        ),
        "ANT_WRAP_DIFF": Spec(
            body=maxx(Src0 - Src1, Src1 - Src0) + C0,
            reference=lambda in0, in1, s0, s1, imm2: np.abs(
                in0.astype(np.float32) - in1.astype(np.float32)
            )
            + s0,
        ),
        "ANT_ABSDIFF_SQ_ACC": Spec(
            body=sq(maxx(Src0, Zero - Src0) - maxx(Src1, Zero - Src1)),
            accum=_op_add,
            accum_init=Zero,
            reference=lambda in0, in1, s0, s1, imm2: (
                lambda b: (b, b.reshape(b.shape[0], -1).sum(-1, keepdims=True))
            )(
                (
                    (np.abs(in0.astype(np.float32)) - np.abs(in1.astype(np.float32)))
                    ** 2
                ).astype(np.float32)
            ),
        ),
    }
    ops = {}
    for name, spec in specs.items():
        if name in dve_ops._SUB_OPCODE_FOR_NAME:
            ops[name] = next(o for o in dve_ops.OPS if o.name == name)
            continue
        row = dve_ops._CUSTOM_DVE_ROW_BASE + len(dve_ops.OPS)
        assert row < 0x20
        shas = {}
        for ver in ("v3", "v4"):
            uops = lower(spec, ver=ver)
            shas[ver] = DveOpSpec(name=name, opcode=row, uops=uops, rd1_en=True).sha(ver)
        op = dve_ops.DveOp(name, spec, subdim=False, uops_sha=shas)
        dve_ops.OPS.append(op)
        dve_ops._SUB_OPCODE_FOR_NAME[name] = row
        dve_ops.CUSTOM_DVE_SPECS[name] = spec
        ops[name] = op
    _CACHE["ops"] = ops
    return ops


def _ap(base, off, dims):
    import concourse.bass as bass

    p_dim = list(base.ap)[0]
    return bass.AP(base.tensor, base.offset + off, [list(p_dim)] + [list(d) for d in dims])


# RECIP approx constants (from concourse.dve_ops RECIP_APPROX_FAST_CONSTS)
_RC0 = -0.23549792
_RC1 = 2.0017324


def _build_nc(repeat=1):
    P = dict(PARTS)
    import concourse.mybir as mybir
    import concourse.tile as tile
    from concourse import bacc
    from contextlib import ExitStack

    OPS = _register_custom_ops()
    f32 = mybir.dt.float32
    bf16 = mybir.dt.bfloat16
    A = mybir.AluOpType
    ACT = mybir.ActivationFunctionType

    nc = bacc.Bacc()

    xyp_d = nc.declare_dram_parameter("xyp", [128, NT * K * 2], bf16, isOutput=False)
    xyg_d = nc.declare_dram_parameter("xyg", [128, NT * K * 2], bf16, isOutput=False)
    cpos_d = nc.declare_dram_parameter("cpos", [128, K], f32, isOutput=False)
    out_d = nc.declare_dram_parameter("partials", [128, 8], f32, isOutput=True)

    with tile.TileContext(nc) as tc:
        with ExitStack() as ctx:
            ep = ctx.enter_context
            p_in = ep(tc.tile_pool(name="inp", bufs=1))
            p_small = ep(tc.tile_pool(name="small", bufs=1))
            p_tbl = ep(tc.tile_pool(name="tbl", bufs=1))
            p_big = ep(tc.tile_pool(name="big", bufs=1))
            p_tiny = ep(tc.tile_pool(name="tiny", bufs=1))
            p_out = ep(tc.tile_pool(name="out", bufs=1))

            xy = {
                "p": p_in.tile([128, NT * K * 2], bf16, name="xyp_t", tag="xyp"),
                "g": p_in.tile([128, NT * K * 2], bf16, name="xyg_t", tag="xyg"),
            }
            nc.sync.dma_start(xy["p"][:], xyp_d[:, :])
            nc.sync.dma_start(xy["g"][:], xyg_d[:, :])
            cpos = p_in.tile([128, K], f32, name="cpos_t", tag="cpos")
            nc.sync.dma_start(cpos[:], cpos_d[:, :])

            outt = p_out.tile([128, 8], f32, name="outt", tag="outt")

            phT = {
                "p": p_tbl.tile([128, TBL], bf16, name="phT_p", tag="phT_p"),
                "g": p_tbl.tile([128, TBL], bf16, name="phT_g", tag="phT_g"),
            }
            for x in ("p", "g"):
                nc.vector.memset(phT[x][:], 0.0)
            b12 = p_tiny.tile([128, 1], f32, name="b12", tag="b12")
            nc.vector.memset(b12[:], 1e-12)
            bnc = p_tiny.tile([128, 1], f32, name="bnc", tag="bnc")
            nc.vector.memset(bnc[:], -PI / 2.0)
            bm1 = p_tiny.tile([128, 1], f32, name="bm1", tag="bm1")
            nc.vector.memset(bm1[:], -1.0)

            # u buffers hold the 8 wrap slices per x (u = |dphi| - pi/2 on the
            # V path, tw = ||dphi| - pi/2| on the S path; ACC abs's both)
            u_p = p_big.tile([128, 8 * DSLICE], bf16, name="u_p", tag="u_p")
            u_g = p_big.tile([128, 8 * DSLICE], bf16, name="u_g", tag="u_g")

            rep_ctx = tc.For_i(0, repeat, 1) if repeat > 1 else None
            if rep_ctx is not None:
                rep_ctx.__enter__()

            # ---------------- small domain ----------------
            if not P['front']:
                nmat = nsums = targ = None
            nmat = {}
            nsums = {}
            targ = {}
            dxm = {}
            dym = {}
            for x in (("p", "g") if P['front'] else ()):
                base = xy[x][:]
                dx = p_small.tile([128, TK], bf16, name=f"dx_{x}", tag=f"dx_{x}")
                dy = p_small.tile([128, TK], bf16, name=f"dy_{x}", tag=f"dy_{x}")
                out_dims = [[KK, NT], [K, K], [1, K]]
                in_i = _ap(base, 0, [[2 * K, NT], [0, K], [2, K]])
                in_j = _ap(base, 0, [[2 * K, NT], [2, K], [0, K]])
                nc.vector.tensor_tensor(_ap(dx[:], 0, out_dims), in_i, in_j, A.subtract)
                in_i = _ap(base, 1, [[2 * K, NT], [0, K], [2, K]])
                in_j = _ap(base, 1, [[2 * K, NT], [2, K], [0, K]])
                nc.vector.tensor_tensor(_ap(dy[:], 0, out_dims), in_i, in_j, A.subtract)
                nsq = p_small.tile([128, TK], bf16, name=f"nsq_{x}", tag=f"nsq_{x}")
                nc.vector._custom_dve(OPS["ANT_SQ_SUM"], out=nsq[:], in0=dx[:], in1=dy[:])

                n_ = p_small.tile([128, TK], bf16, name=f"n_{x}", tag=f"n_{x}")
                ns = p_tiny.tile([128, NT], f32, name=f"nsums_{x}", tag=f"nsums_{x}")
                nc.scalar.activation(n_[:], nsq[:], ACT.Sqrt, bias=b12[:])
                nc.vector.tensor_reduce(
                    ns[:], n_[:].rearrange("p (t k) -> p t k", k=KK),
                    axis=mybir.AxisListType.X, op=A.add,
                )
                nmat[x] = n_
                nsums[x] = ns
                dxm[x] = dx
                dym[x] = dy
            for x in (("p", "g") if P['front'] else ()):
                den = p_small.tile([128, TK], bf16, name=f"den_{x}", tag=f"nsq_{x}")
                nc.vector.tensor_tensor(den[:], nmat[x][:], dxm[x][:], A.add)
                tg = p_small.tile([128, TK], bf16, name=f"targ_{x}", tag=f"targ_{x}")
                nc.vector._custom_dve(
                    OPS["ANT_RECIP_MUL"], out=tg[:], in0=den[:], in1=dym[x][:],
                    s0=_RC0, s1=_RC1, imm2=1e-10,
                )
                targ[x] = tg
            # base-loss V front here to absorb the arctan latency
            df = p_small.tile([128, NT * K * 2], bf16, name="df", tag="df")
            if P['base']:
                nc.vector.tensor_tensor(df[:], xy["p"][:], xy["g"][:], A.subtract)
            inv = {}
            for x in (("p", "g") if P['ratio'] else ()):
                pm = p_tiny.tile([128, NT], f32, name=f"pm_{x}", tag=f"pm_{x}")
                nc.vector.tensor_scalar(pm[:], nsums[x][:], 1.0 / 272.0, 1e-6, A.mult, A.add)
                iv = p_tiny.tile([128, NT], f32, name=f"inv_{x}", tag=f"inv_{x}")
                nc.vector.reciprocal_approx_fast(out=iv[:], in_=pm[:])
                inv[x] = iv

            # ---------------- arctans (after all sqrts) ----------------
            for x in (("p", "g") if P['atan'] else ()):
                tg = targ[x]
                in_full = _ap(tg[:], 0, [[KK, NT], [K, K], [1, K]])
                out_full = _ap(phT[x][:], 0, [[TBL_T, NT], [1, K], [J, K]])
                nc.scalar.activation(out_full, in_full, ACT.Arctan)
                in_wrap = _ap(tg[:], 0, [[KK, NT], [K, K], [1, 8]])
                out_wrap = _ap(phT[x][:], K * J, [[TBL_T, NT], [1, K], [J, 8]])
                nc.scalar.activation(out_wrap, in_wrap, ACT.Arctan)

            # ---------------- big domain wrap slices ----------------
            for x, ubuf in ((("p", u_p), ("g", u_g)) if P['wrap'] else ()):
                for d in range(8):
                    usl = ubuf[:, d * DSLICE : (d + 1) * DSLICE]
                    in0 = _ap(phT[x][:], J * (d + 1), [[TBL_T, NT], [1, K * J]])
                    in1 = _ap(phT[x][:], 0, [[TBL_T, NT], [1, K * J]])
                    if (x, d) in S_PATH:
                        # delta on V, two Abs passes on ScalarE
                        nc.vector.tensor_tensor(usl, in0, in1, A.subtract)
                        nc.scalar.activation(usl, usl, ACT.Abs)
                        nc.scalar.activation(usl, usl, ACT.Abs, bias=bnc[:])
                    else:
                        nc.vector._custom_dve(
                            OPS["ANT_WRAP_DIFF"], out=usl, in0=in0, in1=in1,
                            s0=-PI / 2.0,
                        )
            # accumulate (|u_g| - |u_p|)^2 in two chunks
            H = 4 * DSLICE
            for ci in (range(2) if P['acc'] else ()):
                sl = slice(ci * H, (ci + 1) * H)
                nc.vector._custom_dve(
                    OPS["ANT_ABSDIFF_SQ_ACC"],
                    out=u_g[:, sl], in0=u_g[:, sl], in1=u_p[:, sl],
                    accum_out=outt[:, (0 if ci == 0 else 5) : (1 if ci == 0 else 6)],
                )

            # ---------------- plane correction (one custom op) ----------------
            pj = p_small.tile([128, DSLICE], bf16, name="pj", tag="pj")
            if P['plane']: nc.vector._custom_dve(
                OPS["ANT_ABSDIFF_SQ_ACC"],
                out=pj[:],
                in0=_ap(phT["p"][:], 0, [[TBL_T, NT], [1, K * J]]),
                in1=_ap(phT["g"][:], 0, [[TBL_T, NT], [1, K * J]]),
                accum_out=outt[:, 1:2],
            )

            # ---------------- ratio loss ----------------
            gr = p_small.tile([128, TK], bf16, name="gr", tag="gr")
            xr = p_small.tile([128, TK], bf16, name="xr", tag="xr")
            if P['ratio']:
                nc.vector.tensor_tensor(
                    _ap(gr[:], 0, [[KK, NT], [1, KK]]),
                    _ap(nmat["g"][:], 0, [[KK, NT], [1, KK]]),
                    _ap(inv["g"][:], 0, [[1, NT], [0, KK]]),
                    A.mult,
                )
                nc.vector.tensor_tensor(
                    _ap(xr[:], 0, [[KK, NT], [1, KK]]),
                    _ap(nmat["p"][:], 0, [[KK, NT], [1, KK]]),
                    _ap(inv["p"][:], 0, [[1, NT], [0, KK]]),
                    A.mult,
                )
                nc.vector.tensor_tensor(xr[:], xr[:], gr[:], A.subtract)
            if P['ratio']:
                ax = p_small.tile([128, TK], bf16, name="ax", tag="gr")  # reuse gr
                nc.scalar.activation(ax[:], xr[:], ACT.Abs)
                mn = p_small.tile([128, TK], bf16, name="mn", tag="xr")  # reuse xr
                nc.vector.tensor_single_scalar(mn[:], ax[:], 1.0, A.min)
                nc.scalar.activation(mn[:], mn[:], ACT.Square, accum_out=outt[:, 2:3])
                nc.scalar.activation(ax[:], ax[:], ACT.Relu, bias=bm1[:], accum_out=outt[:, 3:4])

            # ---------------- base loss ----------------
            if P['base']: nc.scalar.activation(df[:], df[:], ACT.Square)
            dsum = p_tiny.tile([128, NT * K], f32, name="dsum", tag="dsum")
            if P['base']: nc.vector.tensor_reduce(
                dsum[:],
                df[:].rearrange("p (a c) -> p a c", c=2),
                axis=mybir.AxisListType.X,
                op=A.add,
            )
            e_ = p_tiny.tile([128, NT * K], f32, name="e_", tag="e")
            if P['base']: nc.vector.tensor_tensor(
                _ap(e_[:], 0, [[K, NT], [1, K]]),
                _ap(dsum[:], 0, [[K, NT], [1, K]]),
                _ap(cpos[:], 0, [[0, NT], [1, K]]),
                A.mult,
            )
            s_ = p_tiny.tile([128, NT * K], f32, name="s_", tag="s")
            if P['base']:
                nc.scalar.activation(s_[:], e_[:], ACT.Sigmoid)
                nc.vector.reciprocal_approx_fast(out=e_[:], in_=s_[:])
                nc.vector.tensor_reduce(
                    outt[:, 4:5], e_[:], axis=mybir.AxisListType.X, op=A.add
                )

            if rep_ctx is not None:
                rep_ctx.__exit__(None, None, None)

            nc.vector.memset(outt[:, 6:8], 0.0)
            nc.sync.dma_start(out_d[:, :], outt[:])

    nc.compile()
    return nc


def _get_nc(repeat=1):
    key = ("nc", repeat)
    if key not in _CACHE:
        _CACHE[key] = _build_nc(repeat)
    return _CACHE[key]


def _host_combine(partials_list):
    ang = sp = rsq = rrelu = bsum = 0.0
    for p in partials_list:
        p = np.asarray(p, dtype=np.float64)
        ang += p[:, 0].sum() + p[:, 5].sum()
        sp += p[:, 1].sum()
        rsq += p[:, 2].sum()
        rrelu += p[:, 3].sum()
        bsum += p[:, 4].sum()
    cnt = float(N * K * (K - 1) * (K - 2))
    angle = 8.0 * (ang - sp) / cnt
    ratio = (0.5 * rsq + rrelu) / 2.0 / 136.0 / N
    base = 2.0 - bsum / (N * K)
    return np.float32(W_BASE * base + W_RATIO * ratio + W_ANGLE * angle)


def _prep_core_inputs(pred, gt, sigmas):
    import ml_dtypes

    bf16 = ml_dtypes.bfloat16
    cpos = (1.0 / (8.0 * np.float64(np.asarray(sigmas)) ** 2)).astype(np.float32)
    cpos_rep = np.ascontiguousarray(np.broadcast_to(cpos[None, :], (128, K)))

    def lay(a):  # [1024,17,3] -> [128, 272] bf16
        b = a[:, :, :2].reshape(NT, 128, K * 2).transpose(1, 0, 2).reshape(128, NT * K * 2)
        return np.ascontiguousarray(b.astype(bf16))

    in_maps = []
    for r in range(N_CORES):
        rows = slice(r * NLOC, (r + 1) * NLOC)
        in_maps.append({"xyp": lay(pred[rows]), "xyg": lay(gt[rows]), "cpos": cpos_rep})
    return in_maps


def run_on_device(pred, gt, sigmas, trace=False):
    from concourse import bass_utils

    nc = _get_nc()
    in_maps = _prep_core_inputs(pred, gt, sigmas)
    res = bass_utils.run_bass_kernel_spmd(nc, in_maps, list(range(N_CORES)), trace=trace)
    partials = [res.results[r]["partials"] for r in range(N_CORES)]
    return _host_combine(partials), res


def _make_fn(nc, in_maps):
    import jax
    from jax.sharding import Mesh, PartitionSpec
    from jax.experimental.shard_map import shard_map
    from concourse import bass2jax, mybir

    bass2jax.install_neuronx_cc_hook()

    part_name = nc.partition_id_tensor.name if nc.partition_id_tensor else None
    in_names, out_names, out_avals, zero_outs = [], [], [], []
    for alloc in nc.m.functions[0].allocations:
        if not isinstance(alloc, mybir.MemoryLocationSet):
            continue
        name = alloc.memorylocations[0].name
        if alloc.kind == "ExternalInput":
            if name != part_name:
                in_names.append(name)
        elif alloc.kind == "ExternalOutput":
            out_names.append(name)
            shape = tuple(alloc.tensor_shape)
            dtype = mybir.dt.np(alloc.dtype)
            out_avals.append(jax.core.ShapedArray(shape, dtype))
            zero_outs.append(np.zeros(shape, dtype))
    n_params = len(in_names)
    n_outs = len(out_avals)
    all_names = in_names + out_names
    if part_name is not None:
        all_names = all_names + [part_name]

    def _body(*args):
        operands = list(args)
        if part_name is not None:
            operands.append(bass2jax.partition_id_tensor())
        outs = bass2jax._bass_exec_p.bind(
            *operands,
            out_avals=tuple(out_avals),
            in_names=tuple(all_names),
            out_names=tuple(out_names),
            lowering_input_output_aliases=(),
            sim_require_finite=True,
            sim_require_nnan=True,
            nc=nc,
        )
        return tuple(outs)

    devices = jax.devices()[:N_CORES]
    mesh = Mesh(np.asarray(devices), ("core",))
    specs = (PartitionSpec("core"),) * (n_params + n_outs)
    out_specs = (PartitionSpec("core"),) * n_outs
    fn = jax.jit(
        shard_map(_body, mesh=mesh, in_specs=specs, out_specs=out_specs, check_rep=False),
        keep_unused=True,
    )
    concat_in = [
        np.concatenate([np.asarray(in_maps[c][nm]) for c in range(N_CORES)], axis=0)
        for nm in in_names
    ]
    concat_zeros = [
        np.zeros((N_CORES * z.shape[0], *z.shape[1:]), z.dtype) for z in zero_outs
    ]
    sharding = jax.sharding.NamedSharding(mesh, PartitionSpec("core"))
    dev_in = [jax.device_put(a, sharding) for a in concat_in]
    dev_zeros = [jax.device_put(a, sharding) for a in concat_zeros]

    def call():
        out = fn(*dev_in, *dev_zeros)
        jax.block_until_ready(out)
        return out

    def read_loss(out):
        out_np = [np.asarray(o) for o in out]
        partials = [
            {nm: out_np[i].reshape(N_CORES, *out_avals[i].shape)[c]
             for i, nm in enumerate(out_names)}
            for c in range(N_CORES)
        ]
        return _host_combine([p["partials"] for p in partials])

    return call, read_loss


def bench_device(pred, gt, sigmas, iters=12, repeat=513, repeat_lo=257):
    """Per-body device time from two large on-device repeat loops, timed
    interleaved so RPC latency drift cancels:
    (T(repeat) - T(repeat_lo)) / (repeat - repeat_lo)."""
    import time

    in_maps = _prep_core_inputs(pred, gt, sigmas)
    call1, read1 = _make_fn(_get_nc(1), in_maps)
    callL, _ = _make_fn(_get_nc(repeat_lo), in_maps)
    callR, _ = _make_fn(_get_nc(repeat), in_maps)

    outs1 = call1()
    loss = read1(outs1)
    callL(); callR()

    sl, sr = [], []
    for _ in range(iters):
        t0 = time.perf_counter(); callL(); sl.append(time.perf_counter() - t0)
        t0 = time.perf_counter(); callR(); sr.append(time.perf_counter() - t0)
    tL = float(np.median(sl)); tR = float(np.median(sr))
    per_iter_ns = (tR - tL) / (repeat - repeat_lo) * 1e9
    return per_iter_ns, loss, tL * 1e9, tR * 1e9


def _numpy_fallback(pred_kpts, gt_kpts, kpt_mask, sigmas):
    p = np.asarray(pred_kpts, np.float64)
    g = np.asarray(gt_kpts, np.float64)
    mask = np.asarray(kpt_mask, np.float64)
    sig = np.asarray(sigmas, np.float64)
    n_, k_ = mask.shape

    d = (p[..., 0] - g[..., 0]) ** 2 + (p[..., 1] - g[..., 1]) ** 2
    factor = k_ / ((mask != 0).sum(1) + 1e-9)
    e = d / ((2.0 * sig) ** 2 * 2.0)
    base = np.mean(factor[:, None] * ((1.0 - np.exp(-e)) * mask))

    vm = (mask > 0).astype(np.float64)
    pxy, gxy = p[..., :2], g[..., :2]

    def pdist(x):
        diff = x[:, :, None, :] - x[:, None, :, :]
        return np.sqrt(np.maximum((diff * diff).sum(-1), 0.0))

    iu = np.triu(np.ones((k_, k_)), k=1)
    pairm = vm[:, :, None] * vm[:, None, :] * iu[None]
    npairs = pairm.sum((1, 2))
    denom = np.maximum(npairs, 1.0)
    pd_, gd_ = pdist(pxy), pdist(gxy)
    pmean = (pd_ * pairm).sum((1, 2)) / denom
    gmean = (gd_ * pairm).sum((1, 2)) / denom
    prr = pd_ / (pmean + 1e-6)[:, None, None]
    grr = gd_ / (gmean + 1e-6)[:, None, None]
    x = prr - grr
    ax = np.abs(x)
    sm = np.where(ax < 1.0, 0.5 * x * x, ax - 0.5)
    rl = (sm * pairm).sum((1, 2)) / denom
    valid = (npairs >= 1).astype(np.float64)
    ratio = (rl * valid).sum() / max(valid.sum(), 1.0)

    def angles(x):
        D = x[:, None, :, :] - x[:, :, None, :]
        dot = np.einsum("bjid,bjkd->bjik", D, D)
        nn = np.sqrt(np.maximum((D * D).sum(-1), 0.0))
        den = nn[:, :, :, None] * nn[:, :, None, :] + 1e-6
        return np.arccos(np.clip(dot / den, -1.0, 1.0))

    ap_, ag_ = angles(pxy), angles(gxy)
    ne = ~np.eye(k_, dtype=bool)
    trip = (ne[:, :, None] & ne[:, None, :] & ne[None, :, :]).astype(np.float64)
    tm = vm[:, :, None, None] * vm[:, None, :, None] * vm[:, None, None, :] * trip[None]
    cnt = tm.sum()
    angle = (((ap_ - ag_) ** 2) * tm).sum() / max(cnt, 1.0)
    return np.float32(W_BASE * base + W_RATIO * ratio + W_ANGLE * angle)


def _persistent_call(pred, gt, sig):
    """Jit the 8-core executable once; per call only upload fresh inputs."""
    import jax
    from jax.sharding import Mesh, PartitionSpec

    in_maps = _prep_core_inputs(pred, gt, sig)
    ent = _CACHE.get("persist")
    if ent is None:
        fn, in_names, out_names, out_avals, mesh = _make_raw_fn(_get_nc(1))
        sharding = jax.sharding.NamedSharding(mesh, PartitionSpec("core"))
        zero_outs = [
            np.zeros((N_CORES * a.shape[0], *a.shape[1:]), a.dtype) for a in out_avals
        ]
        dev_zeros = [jax.device_put(z, sharding) for z in zero_outs]
        ent = (fn, in_names, out_names, out_avals, sharding, dev_zeros)
        _CACHE["persist"] = ent
    fn, in_names, out_names, out_avals, sharding, dev_zeros = ent
    concat_in = [
        np.concatenate([np.asarray(in_maps[c][nm]) for c in range(N_CORES)], axis=0)
        for nm in in_names
    ]
    dev_in = [jax.device_put(a, sharding) for a in concat_in]
    out = fn(*dev_in, *dev_zeros)
    jax.block_until_ready(out)
    out_np = [np.asarray(o) for o in out]
    partials = [
        out_np[out_names.index("partials")].reshape(N_CORES, 128, -1)[c]
        for c in range(N_CORES)
    ]
    return _host_combine(partials)


def _make_raw_fn(nc):
    """jitted shard_map callable + metadata (no baked-in inputs)."""
    import jax
    from jax.sharding import Mesh, PartitionSpec
    from jax.experimental.shard_map import shard_map
    from concourse import bass2jax, mybir

    bass2jax.install_neuronx_cc_hook()
    part_name = nc.partition_id_tensor.name if nc.partition_id_tensor else None
    in_names, out_names, out_avals = [], [], []
    for alloc in nc.m.functions[0].allocations:
        if not isinstance(alloc, mybir.MemoryLocationSet):
            continue
        name = alloc.memorylocations[0].name
        if alloc.kind == "ExternalInput":
            if name != part_name:
                in_names.append(name)
        elif alloc.kind == "ExternalOutput":
            out_names.append(name)
            shape = tuple(alloc.tensor_shape)
            dtype = mybir.dt.np(alloc.dtype)
            out_avals.append(jax.core.ShapedArray(shape, dtype))
    all_names = in_names + out_names
    if part_name is not None:
        all_names = all_names + [part_name]

    def _body(*args):
        operands = list(args)
        if part_name is not None:
            operands.append(bass2jax.partition_id_tensor())
        outs = bass2jax._bass_exec_p.bind(
            *operands,
            out_avals=tuple(out_avals),
            in_names=tuple(all_names),
            out_names=tuple(out_names),
            lowering_input_output_aliases=(),
            sim_require_finite=True,
            sim_require_nnan=True,
            nc=nc,
        )
        return tuple(outs)

    devices = jax.devices()[:N_CORES]
    mesh = Mesh(np.asarray(devices), ("core",))
    n_params, n_outs = len(in_names), len(out_avals)
    specs = (PartitionSpec("core"),) * (n_params + n_outs)
    out_specs = (PartitionSpec("core"),) * n_outs
    fn = jax.jit(
        shard_map(_body, mesh=mesh, in_specs=specs, out_specs=out_specs, check_rep=False),
        keep_unused=True,
    )
    return fn, in_names, out_names, out_avals, mesh


def kernel(pred_kpts, gt_kpts, kpt_mask, sigmas):
    pred = np.asarray(pred_kpts, dtype=np.float32)
    gt = np.asarray(gt_kpts, dtype=np.float32)
    mask = np.asarray(kpt_mask, dtype=np.float32)
    sig = np.asarray(sigmas, dtype=np.float32)
    if pred.shape != (N, K, 3) or not np.all(mask == 1.0):
        return _numpy_fallback(pred, gt, mask, sig)
    loss = _persistent_call(pred, gt, sig)
    return loss


# revision 10
# speedup vs baseline: 1.0596x; 1.0596x over previous
"""Trainium2 Bass kernel v3 for nn_EnhancedKeypointLoss.

Key techniques:
  - 2 ScalarE table loads total (all Sqrts, then Arctan/Sigmoid set; the
    base-loss exp is computed via exp(-e) = 1/sigmoid(e) - 1).
  - Signed half-angle phi = 2*atan(dy/(n+dx)); HW arctan table is accurate
    over the full fp32 range, so no quadrant fixup.
  - bf16 throughout the bulk domains; 8 row-tiles batched per instruction.
  - Custom DVE ops (registered at import):
      ANT_SQ_SUM:   out = Src0^2 + Src1^2            (nsq in one pass)
      ANT_RECIP_MUL:out = Src1 * approx(1/Src0)      (seed + 1 NR step)
      ANT_WRAP_DIFF:out = |Src0 - Src1| + C0         (pair delta + wrap)
      ANT_ABSDIFF_SQ_ACC: out=(|Src0|-|Src1|)^2, accum=sum
  - Wrapped-angle big domain [8 tiles x 8 shifts x 17 x 18] with the phi
    table transposed and j-padded to 18 (pad column zeroed -> terms cancel).
  - V/S engine balance: a subset of the 16 (x, d) wrap slices is routed
    through ScalarE (two Abs passes) instead of the VectorE custom op.
"""

import numpy as np
from operator import add as _op_add

N_CORES = 8
N = 8192
K = 17
NLOC = N // N_CORES  # 1024
NT = NLOC // 128  # 8 tiles per core
KK = K * K  # 289
TK = NT * KK  # 2312
J = 17  # j dimension (no padding: custom DVE ops run 1x, no alignment constraint)
I25 = 25  # i rows incl. 8 wrap rows
TBL_T = I25 * J  # 450 per tile
TBL = NT * TBL_T  # 3600
DSLICE = NT * K * J  # 2448 elements per (x, d) wrap slice
PI = float(np.float32(np.pi))
W_BASE, W_RATIO, W_ANGLE = 1.0, 0.2, 0.2

# (x, d) slices routed through the ScalarE abs-chain instead of the V custom
S_PATH = {('g', 1), ('g', 3), ('g', 5), ('g', 7)}

_CACHE = {}
PARTS = {'front': True, 'atan': True, 'wrap': True, 'acc': True, 'plane': True, 'ratio': True, 'base': True}


def _register_custom_ops():
    if "ops" in _CACHE:
        return _CACHE["ops"]
    import concourse.dve_ops as dve_ops
    from concourse.dve_spec import Spec, Src0, Src1, C0, C1, C2, Bin, AluOp, maxx, sq, lower, Zero
    from concourse.dve_uop import DveOpSpec

    _xc = maxx(Src0, C2)
    _notx = Bin(AluOp.BITWISE_NOT, _xc, _xc)
    _y0 = _notx * C0
    specs = {
        "ANT_SQ_SUM": Spec(
            body=sq(Src0) + sq(Src1),
            reference=lambda in0, in1, s0, s1, imm2: (
                in0.astype(np.float32) ** 2 + in1.astype(np.float32) ** 2
            ),
        ),
        "ANT_RECIP_MUL": Spec(
            body=(_y0 * (C1 - _xc * _y0)) * Src1,
            reference=lambda in0, in1, s0, s1, imm2: (
                (lambda xc: (lambda y0: (y0 * (s1 - xc * y0)) * in1)(
                    (~xc.view(np.int32)).view(np.float32) * s0
                ))(np.maximum(in0.astype(np.float32), imm2))
            ),
        ),
        "ANT_WRAP_DIFF": Spec(
            body=maxx(Src0 - Src1, Src1 - Src0) + C0,
            reference=lambda in0, in1, s0, s1, imm2: np.abs(
                in0.astype(np.float32) - in1.astype(np.float32)
            )
            + s0,
        ),
        "ANT_ABSDIFF_SQ_ACC": Spec(
            body=sq(maxx(Src0, Zero - Src0) - maxx(Src1, Zero - Src1)),
            accum=_op_add,
            accum_init=Zero,
            reference=lambda in0, in1, s0, s1, imm2: (
                lambda b: (b, b.reshape(b.shape[0], -1).sum(-1, keepdims=True))
            )(
                (
                    (np.abs(in0.astype(np.float32)) - np.abs(in1.astype(np.float32)))
                    ** 2
                ).astype(np.float32)
            ),
        ),
    }
    ops = {}
    for name, spec in specs.items():
        if name in dve_ops._SUB_OPCODE_FOR_NAME:
            ops[name] = next(o for o in dve_ops.OPS if o.name == name)
            continue
        row = dve_ops._CUSTOM_DVE_ROW_BASE + len(dve_ops.OPS)
        assert row < 0x20
        shas = {}
        for ver in ("v3", "v4"):
            uops = lower(spec, ver=ver)
            shas[ver] = DveOpSpec(name=name, opcode=row, uops=uops, rd1_en=True).sha(ver)
        op = dve_ops.DveOp(name, spec, subdim=False, uops_sha=shas)
        dve_ops.OPS.append(op)
        dve_ops._SUB_OPCODE_FOR_NAME[name] = row
        dve_ops.CUSTOM_DVE_SPECS[name] = spec
        ops[name] = op
    _CACHE["ops"] = ops
    return ops


def _ap(base, off, dims):
    import concourse.bass as bass

    p_dim = list(base.ap)[0]
    return bass.AP(base.tensor, base.offset + off, [list(p_dim)] + [list(d) for d in dims])


# RECIP approx constants (from concourse.dve_ops RECIP_APPROX_FAST_CONSTS)
_RC0 = -0.23549792
_RC1 = 2.0017324


def _build_nc(repeat=1):
    P = dict(PARTS)
    import concourse.mybir as mybir
    import concourse.tile as tile
    from concourse import bacc
    from contextlib import ExitStack

    OPS = _register_custom_ops()
    f32 = mybir.dt.float32
    bf16 = mybir.dt.bfloat16
    A = mybir.AluOpType
    ACT = mybir.ActivationFunctionType

    nc = bacc.Bacc()

    xyp_d = nc.declare_dram_parameter("xyp", [128, NT * K * 2], bf16, isOutput=False)
    xyg_d = nc.declare_dram_parameter("xyg", [128, NT * K * 2], bf16, isOutput=False)
    cpos_d = nc.declare_dram_parameter("cpos", [128, K], f32, isOutput=False)
    out_d = nc.declare_dram_parameter("partials", [128, 8], f32, isOutput=True)

    with tile.TileContext(nc) as tc:
        with ExitStack() as ctx:
            ep = ctx.enter_context
            p_in = ep(tc.tile_pool(name="inp", bufs=1))
            p_small = ep(tc.tile_pool(name="small", bufs=1))
            p_tbl = ep(tc.tile_pool(name="tbl", bufs=1))
            p_big = ep(tc.tile_pool(name="big", bufs=1))
            p_tiny = ep(tc.tile_pool(name="tiny", bufs=1))
            p_out = ep(tc.tile_pool(name="out", bufs=1))

            xy = {
                "p": p_in.tile([128, NT * K * 2], bf16, name="xyp_t", tag="xyp"),
                "g": p_in.tile([128, NT * K * 2], bf16, name="xyg_t", tag="xyg"),
            }
            nc.sync.dma_start(xy["p"][:], xyp_d[:, :])
            nc.sync.dma_start(xy["g"][:], xyg_d[:, :])
            cpos = p_in.tile([128, K], f32, name="cpos_t", tag="cpos")
            nc.sync.dma_start(cpos[:], cpos_d[:, :])

            outt = p_out.tile([128, 8], f32, name="outt", tag="outt")

            phT = {
                "p": p_tbl.tile([128, TBL], bf16, name="phT_p", tag="phT_p"),
                "g": p_tbl.tile([128, TBL], bf16, name="phT_g", tag="phT_g"),
            }
            for x in ("p", "g"):
                nc.vector.memset(phT[x][:], 0.0)
            b12 = p_tiny.tile([128, 1], f32, name="b12", tag="b12")
            nc.vector.memset(b12[:], 1e-12)
            bnc = p_tiny.tile([128, 1], f32, name="bnc", tag="bnc")
            nc.vector.memset(bnc[:], -PI / 2.0)
            bm1 = p_tiny.tile([128, 1], f32, name="bm1", tag="bm1")
            nc.vector.memset(bm1[:], -1.0)

            # u buffers hold the 8 wrap slices per x (u = |dphi| - pi/2 on the
            # V path, tw = ||dphi| - pi/2| on the S path; ACC abs's both)
            u_p = p_big.tile([128, 8 * DSLICE], bf16, name="u_p", tag="u_p")
            u_g = p_big.tile([128, 8 * DSLICE], bf16, name="u_g", tag="u_g")

            rep_ctx = tc.For_i(0, repeat, 1) if repeat > 1 else None
            if rep_ctx is not None:
                rep_ctx.__enter__()

            # ---------------- small domain ----------------
            if not P['front']:
                nmat = nsums = targ = None
            nmat = {}
            nsums = {}
            targ = {}
            dxm = {}
            dym = {}
            for x in (("p", "g") if P['front'] else ()):
                base = xy[x][:]
                dx = p_small.tile([128, TK], bf16, name=f"dx_{x}", tag=f"dx_{x}")
                dy = p_small.tile([128, TK], bf16, name=f"dy_{x}", tag=f"dy_{x}")
                out_dims = [[KK, NT], [K, K], [1, K]]
                in_i = _ap(base, 0, [[2 * K, NT], [0, K], [2, K]])
                in_j = _ap(base, 0, [[2 * K, NT], [2, K], [0, K]])
                nc.vector.tensor_tensor(_ap(dx[:], 0, out_dims), in_i, in_j, A.subtract)
                in_i = _ap(base, 1, [[2 * K, NT], [0, K], [2, K]])
                in_j = _ap(base, 1, [[2 * K, NT], [2, K], [0, K]])
                nc.vector.tensor_tensor(_ap(dy[:], 0, out_dims), in_i, in_j, A.subtract)
                nsq = p_small.tile([128, TK], bf16, name=f"nsq_{x}", tag=f"nsq_{x}")
                nc.vector._custom_dve(OPS["ANT_SQ_SUM"], out=nsq[:], in0=dx[:], in1=dy[:])

                n_ = p_small.tile([128, TK], bf16, name=f"n_{x}", tag=f"n_{x}")
                ns = p_tiny.tile([128, NT], f32, name=f"nsums_{x}", tag=f"nsums_{x}")
                nc.scalar.activation(n_[:], nsq[:], ACT.Sqrt, bias=b12[:])
                nc.vector.tensor_reduce(
                    ns[:], n_[:].rearrange("p (t k) -> p t k", k=KK),
                    axis=mybir.AxisListType.X, op=A.add,
                )
                nmat[x] = n_
                nsums[x] = ns
                dxm[x] = dx
                dym[x] = dy
            for x in (("p", "g") if P['front'] else ()):
                den = p_small.tile([128, TK], bf16, name=f"den_{x}", tag=f"nsq_{x}")
                nc.vector.tensor_tensor(den[:], nmat[x][:], dxm[x][:], A.add)
                tg = p_small.tile([128, TK], bf16, name=f"targ_{x}", tag=f"targ_{x}")
                nc.vector._custom_dve(
                    OPS["ANT_RECIP_MUL"], out=tg[:], in0=den[:], in1=dym[x][:],
                    s0=_RC0, s1=_RC1, imm2=1e-10,
                )
                targ[x] = tg
            # base-loss V front here to absorb the arctan latency
            df = p_small.tile([128, NT * K * 2], bf16, name="df", tag="df")
            if P['base']:
                nc.vector.tensor_tensor(df[:], xy["p"][:], xy["g"][:], A.subtract)
            inv = {}
            for x in (("p", "g") if P['ratio'] else ()):
                pm = p_tiny.tile([128, NT], f32, name=f"pm_{x}", tag=f"pm_{x}")
                nc.vector.tensor_scalar(pm[:], nsums[x][:], 1.0 / 272.0, 1e-6, A.mult, A.add)
                iv = p_tiny.tile([128, NT], f32, name=f"inv_{x}", tag=f"inv_{x}")
                nc.vector.reciprocal_approx_fast(out=iv[:], in_=pm[:])
                inv[x] = iv

            gr = p_small.tile([128, TK], bf16, name="gr", tag="gr")
            xr = p_small.tile([128, TK], bf16, name="xr", tag="xr")
            if P['ratio']:
                nc.vector.tensor_tensor(
                    _ap(gr[:], 0, [[KK, NT], [1, KK]]),
                    _ap(nmat["g"][:], 0, [[KK, NT], [1, KK]]),
                    _ap(inv["g"][:], 0, [[1, NT], [0, KK]]),
                    A.mult,
                )
                nc.vector.tensor_tensor(
                    _ap(xr[:], 0, [[KK, NT], [1, KK]]),
                    _ap(nmat["p"][:], 0, [[KK, NT], [1, KK]]),
                    _ap(inv["p"][:], 0, [[1, NT], [0, KK]]),
                    A.mult,
                )
                nc.vector.tensor_tensor(xr[:], xr[:], gr[:], A.subtract)

            # ---------------- arctans (after all sqrts) ----------------
            for x in (("p", "g") if P['atan'] else ()):
                tg = targ[x]
                in_full = _ap(tg[:], 0, [[KK, NT], [K, K], [1, K]])
                out_full = _ap(phT[x][:], 0, [[TBL_T, NT], [1, K], [J, K]])
                nc.scalar.activation(out_full, in_full, ACT.Arctan)
                in_wrap = _ap(tg[:], 0, [[KK, NT], [K, K], [1, 8]])
                out_wrap = _ap(phT[x][:], K * J, [[TBL_T, NT], [1, K], [J, 8]])
                nc.scalar.activation(out_wrap, in_wrap, ACT.Arctan)

            # ---------------- big domain wrap slices ----------------
            for x, ubuf in ((("p", u_p), ("g", u_g)) if P['wrap'] else ()):
                for d in range(8):
                    usl = ubuf[:, d * DSLICE : (d + 1) * DSLICE]
                    in0 = _ap(phT[x][:], J * (d + 1), [[TBL_T, NT], [1, K * J]])
                    in1 = _ap(phT[x][:], 0, [[TBL_T, NT], [1, K * J]])
                    if (x, d) in S_PATH:
                        # delta on V, two Abs passes on ScalarE
                        nc.vector.tensor_tensor(usl, in0, in1, A.subtract)
                        nc.scalar.activation(usl, usl, ACT.Abs)
                        nc.scalar.activation(usl, usl, ACT.Abs, bias=bnc[:])
                    else:
                        nc.vector._custom_dve(
                            OPS["ANT_WRAP_DIFF"], out=usl, in0=in0, in1=in1,
                            s0=-PI / 2.0,
                        )
            # accumulate (|u_g| - |u_p|)^2 in two chunks
            H = 4 * DSLICE
            for ci in (range(2) if P['acc'] else ()):
                sl = slice(ci * H, (ci + 1) * H)
                nc.vector._custom_dve(
                    OPS["ANT_ABSDIFF_SQ_ACC"],
                    out=u_g[:, sl], in0=u_g[:, sl], in1=u_p[:, sl],
                    accum_out=outt[:, (0 if ci == 0 else 5) : (1 if ci == 0 else 6)],
                )

            # ---------------- plane correction (one custom op) ----------------
            pj = p_small.tile([128, DSLICE], bf16, name="pj", tag="pj")
            if P['plane']: nc.vector._custom_dve(
                OPS["ANT_ABSDIFF_SQ_ACC"],
                out=pj[:],
                in0=_ap(phT["p"][:], 0, [[TBL_T, NT], [1, K * J]]),
                in1=_ap(phT["g"][:], 0, [[TBL_T, NT], [1, K * J]]),
                accum_out=outt[:, 1:2],
            )

            # ---------------- ratio loss ----------------
            if P['ratio']:
                ax = p_small.tile([128, TK], bf16, name="ax", tag="gr")  # reuse gr
                nc.scalar.activation(ax[:], xr[:], ACT.Abs)
                mn = p_small.tile([128, TK], bf16, name="mn", tag="xr")  # reuse xr
                nc.vector.tensor_single_scalar(mn[:], ax[:], 1.0, A.min)
                nc.scalar.activation(mn[:], mn[:], ACT.Square, accum_out=outt[:, 2:3])
                nc.scalar.activation(ax[:], ax[:], ACT.Relu, bias=bm1[:], accum_out=outt[:, 3:4])

            # ---------------- base loss ----------------
            if P['base']: nc.scalar.activation(df[:], df[:], ACT.Square)
            dsum = p_tiny.tile([128, NT * K], f32, name="dsum", tag="dsum")
            if P['base']: nc.vector.tensor_reduce(
                dsum[:],
                df[:].rearrange("p (a c) -> p a c", c=2),
                axis=mybir.AxisListType.X,
                op=A.add,
            )
            e_ = p_tiny.tile([128, NT * K], f32, name="e_", tag="e")
            if P['base']: nc.vector.tensor_tensor(
                _ap(e_[:], 0, [[K, NT], [1, K]]),
                _ap(dsum[:], 0, [[K, NT], [1, K]]),
                _ap(cpos[:], 0, [[0, NT], [1, K]]),
                A.mult,
            )
            s_ = p_tiny.tile([128, NT * K], f32, name="s_", tag="s")
            if P['base']:
                nc.scalar.activation(s_[:], e_[:], ACT.Sigmoid)
                nc.vector.reciprocal_approx_fast(out=e_[:], in_=s_[:])
                nc.vector.tensor_reduce(
                    outt[:, 4:5], e_[:], axis=mybir.AxisListType.X, op=A.add
                )

            if rep_ctx is not None:
                rep_ctx.__exit__(None, None, None)

            nc.vector.memset(outt[:, 6:8], 0.0)
            nc.sync.dma_start(out_d[:, :], outt[:])

    nc.compile()
    return nc


def _get_nc(repeat=1):
    key = ("nc", repeat)
    if key not in _CACHE:
        _CACHE[key] = _build_nc(repeat)
    return _CACHE[key]


def _host_combine(partials_list):
    ang = sp = rsq = rrelu = bsum = 0.0
    for p in partials_list:
        p = np.asarray(p, dtype=np.float64)
        ang += p[:, 0].sum() + p[:, 5].sum()
        sp += p[:, 1].sum()
        rsq += p[:, 2].sum()
        rrelu += p[:, 3].sum()
        bsum += p[:, 4].sum()
    cnt = float(N * K * (K - 1) * (K - 2))
    angle = 8.0 * (ang - sp) / cnt
    ratio = (0.5 * rsq + rrelu) / 2.0 / 136.0 / N
    base = 2.0 - bsum / (N * K)
    return np.float32(W_BASE * base + W_RATIO * ratio + W_ANGLE * angle)


def _prep_core_inputs(pred, gt, sigmas):
    import ml_dtypes

    bf16 = ml_dtypes.bfloat16
    cpos = (1.0 / (8.0 * np.float64(np.asarray(sigmas)) ** 2)).astype(np.float32)
    cpos_rep = np.ascontiguousarray(np.broadcast_to(cpos[None, :], (128, K)))

    def lay(a):  # [1024,17,3] -> [128, 272] bf16
        b = a[:, :, :2].reshape(NT, 128, K * 2).transpose(1, 0, 2).reshape(128, NT * K * 2)
        return np.ascontiguousarray(b.astype(bf16))

    in_maps = []
    for r in range(N_CORES):
        rows = slice(r * NLOC, (r + 1) * NLOC)
        in_maps.append({"xyp": lay(pred[rows]), "xyg": lay(gt[rows]), "cpos": cpos_rep})
    return in_maps


def run_on_device(pred, gt, sigmas, trace=False):
    from concourse import bass_utils

    nc = _get_nc()
    in_maps = _prep_core_inputs(pred, gt, sigmas)
    res = bass_utils.run_bass_kernel_spmd(nc, in_maps, list(range(N_CORES)), trace=trace)
    partials = [res.results[r]["partials"] for r in range(N_CORES)]
    return _host_combine(partials), res


def _make_fn(nc, in_maps):
    import jax
    from jax.sharding import Mesh, PartitionSpec
    from jax.experimental.shard_map import shard_map
    from concourse import bass2jax, mybir

    bass2jax.install_neuronx_cc_hook()

    part_name = nc.partition_id_tensor.name if nc.partition_id_tensor else None
    in_names, out_names, out_avals, zero_outs = [], [], [], []
    for alloc in nc.m.functions[0].allocations:
        if not isinstance(alloc, mybir.MemoryLocationSet):
            continue
        name = alloc.memorylocations[0].name
        if alloc.kind == "ExternalInput":
            if name != part_name:
                in_names.append(name)
        elif alloc.kind == "ExternalOutput":
            out_names.append(name)
            shape = tuple(alloc.tensor_shape)
            dtype = mybir.dt.np(alloc.dtype)
            out_avals.append(jax.core.ShapedArray(shape, dtype))
            zero_outs.append(np.zeros(shape, dtype))
    n_params = len(in_names)
    n_outs = len(out_avals)
    all_names = in_names + out_names
    if part_name is not None:
        all_names = all_names + [part_name]

    def _body(*args):
        operands = list(args)
        if part_name is not None:
            operands.append(bass2jax.partition_id_tensor())
        outs = bass2jax._bass_exec_p.bind(
            *operands,
            out_avals=tuple(out_avals),
            in_names=tuple(all_names),
            out_names=tuple(out_names),
            lowering_input_output_aliases=(),
            sim_require_finite=True,
            sim_require_nnan=True,
            nc=nc,
        )
        return tuple(outs)

    devices = jax.devices()[:N_CORES]
    mesh = Mesh(np.asarray(devices), ("core",))
    specs = (PartitionSpec("core"),) * (n_params + n_outs)
    out_specs = (PartitionSpec("core"),) * n_outs
    fn = jax.jit(
        shard_map(_body, mesh=mesh, in_specs=specs, out_specs=out_specs, check_rep=False),
        keep_unused=True,
    )
    concat_in = [
        np.concatenate([np.asarray(in_maps[c][nm]) for c in range(N_CORES)], axis=0)
        for nm in in_names
    ]
    concat_zeros = [
        np.zeros((N_CORES * z.shape[0], *z.shape[1:]), z.dtype) for z in zero_outs
    ]
    sharding = jax.sharding.NamedSharding(mesh, PartitionSpec("core"))
    dev_in = [jax.device_put(a, sharding) for a in concat_in]
    dev_zeros = [jax.device_put(a, sharding) for a in concat_zeros]

    def call():
        out = fn(*dev_in, *dev_zeros)
        jax.block_until_ready(out)
        return out

    def read_loss(out):
        out_np = [np.asarray(o) for o in out]
        partials = [
            {nm: out_np[i].reshape(N_CORES, *out_avals[i].shape)[c]
             for i, nm in enumerate(out_names)}
            for c in range(N_CORES)
        ]
        return _host_combine([p["partials"] for p in partials])

    return call, read_loss


def bench_device(pred, gt, sigmas, iters=12, repeat=513, repeat_lo=257):
    """Per-body device time from two large on-device repeat loops, timed
    interleaved so RPC latency drift cancels:
    (T(repeat) - T(repeat_lo)) / (repeat - repeat_lo)."""
    import time

    in_maps = _prep_core_inputs(pred, gt, sigmas)
    call1, read1 = _make_fn(_get_nc(1), in_maps)
    callL, _ = _make_fn(_get_nc(repeat_lo), in_maps)
    callR, _ = _make_fn(_get_nc(repeat), in_maps)

    outs1 = call1()
    loss = read1(outs1)
    callL(); callR()

    sl, sr = [], []
    for _ in range(iters):
        t0 = time.perf_counter(); callL(); sl.append(time.perf_counter() - t0)
        t0 = time.perf_counter(); callR(); sr.append(time.perf_counter() - t0)
    tL = float(np.median(sl)); tR = float(np.median(sr))
    per_iter_ns = (tR - tL) / (repeat - repeat_lo) * 1e9
    return per_iter_ns, loss, tL * 1e9, tR * 1e9


def _numpy_fallback(pred_kpts, gt_kpts, kpt_mask, sigmas):
    p = np.asarray(pred_kpts, np.float64)
    g = np.asarray(gt_kpts, np.float64)
    mask = np.asarray(kpt_mask, np.float64)
    sig = np.asarray(sigmas, np.float64)
    n_, k_ = mask.shape

    d = (p[..., 0] - g[..., 0]) ** 2 + (p[..., 1] - g[..., 1]) ** 2
    factor = k_ / ((mask != 0).sum(1) + 1e-9)
    e = d / ((2.0 * sig) ** 2 * 2.0)
    base = np.mean(factor[:, None] * ((1.0 - np.exp(-e)) * mask))

    vm = (mask > 0).astype(np.float64)
    pxy, gxy = p[..., :2], g[..., :2]

    def pdist(x):
        diff = x[:, :, None, :] - x[:, None, :, :]
        return np.sqrt(np.maximum((diff * diff).sum(-1), 0.0))

    iu = np.triu(np.ones((k_, k_)), k=1)
    pairm = vm[:, :, None] * vm[:, None, :] * iu[None]
    npairs = pairm.sum((1, 2))
    denom = np.maximum(npairs, 1.0)
    pd_, gd_ = pdist(pxy), pdist(gxy)
    pmean = (pd_ * pairm).sum((1, 2)) / denom
    gmean = (gd_ * pairm).sum((1, 2)) / denom
    prr = pd_ / (pmean + 1e-6)[:, None, None]
    grr = gd_ / (gmean + 1e-6)[:, None, None]
    x = prr - grr
    ax = np.abs(x)
    sm = np.where(ax < 1.0, 0.5 * x * x, ax - 0.5)
    rl = (sm * pairm).sum((1, 2)) / denom
    valid = (npairs >= 1).astype(np.float64)
    ratio = (rl * valid).sum() / max(valid.sum(), 1.0)

    def angles(x):
        D = x[:, None, :, :] - x[:, :, None, :]
        dot = np.einsum("bjid,bjkd->bjik", D, D)
        nn = np.sqrt(np.maximum((D * D).sum(-1), 0.0))
        den = nn[:, :, :, None] * nn[:, :, None, :] + 1e-6
        return np.arccos(np.clip(dot / den, -1.0, 1.0))

    ap_, ag_ = angles(pxy), angles(gxy)
    ne = ~np.eye(k_, dtype=bool)
    trip = (ne[:, :, None] & ne[:, None, :] & ne[None, :, :]).astype(np.float64)
    tm = vm[:, :, None, None] * vm[:, None, :, None] * vm[:, None, None, :] * trip[None]
    cnt = tm.sum()
    angle = (((ap_ - ag_) ** 2) * tm).sum() / max(cnt, 1.0)
    return np.float32(W_BASE * base + W_RATIO * ratio + W_ANGLE * angle)


def _persistent_call(pred, gt, sig):
    """Jit the 8-core executable once; per call only upload fresh inputs."""
    import jax
    from jax.sharding import Mesh, PartitionSpec

    in_maps = _prep_core_inputs(pred, gt, sig)
    ent = _CACHE.get("persist")
    if ent is None:
        fn, in_names, out_names, out_avals, mesh = _make_raw_fn(_get_nc(1))
        sharding = jax.sharding.NamedSharding(mesh, PartitionSpec("core"))
        zero_outs = [
            np.zeros((N_CORES * a.shape[0], *a.shape[1:]), a.dtype) for a in out_avals
        ]
        dev_zeros = [jax.device_put(z, sharding) for z in zero_outs]
        ent = (fn, in_names, out_names, out_avals, sharding, dev_zeros)
        _CACHE["persist"] = ent
    fn, in_names, out_names, out_avals, sharding, dev_zeros = ent
    concat_in = [
        np.concatenate([np.asarray(in_maps[c][nm]) for c in range(N_CORES)], axis=0)
        for nm in in_names
    ]
    dev_in = [jax.device_put(a, sharding) for a in concat_in]
    out = fn(*dev_in, *dev_zeros)
    jax.block_until_ready(out)
    out_np = [np.asarray(o) for o in out]
    partials = [
        out_np[out_names.index("partials")].reshape(N_CORES, 128, -1)[c]
        for c in range(N_CORES)
    ]
    return _host_combine(partials)


def _make_raw_fn(nc):
    """jitted shard_map callable + metadata (no baked-in inputs)."""
    import jax
    from jax.sharding import Mesh, PartitionSpec
    from jax.experimental.shard_map import shard_map
    from concourse import bass2jax, mybir

    bass2jax.install_neuronx_cc_hook()
    part_name = nc.partition_id_tensor.name if nc.partition_id_tensor else None
    in_names, out_names, out_avals = [], [], []
    for alloc in nc.m.functions[0].allocations:
        if not isinstance(alloc, mybir.MemoryLocationSet):
            continue
        name = alloc.memorylocations[0].name
        if alloc.kind == "ExternalInput":
            if name != part_name:
                in_names.append(name)
        elif alloc.kind == "ExternalOutput":
            out_names.append(name)
            shape = tuple(alloc.tensor_shape)
            dtype = mybir.dt.np(alloc.dtype)
            out_avals.append(jax.core.ShapedArray(shape, dtype))
    all_names = in_names + out_names
    if part_name is not None:
        all_names = all_names + [part_name]

    def _body(*args):
        operands = list(args)
        if part_name is not None:
            operands.append(bass2jax.partition_id_tensor())
        outs = bass2jax._bass_exec_p.bind(
            *operands,
            out_avals=tuple(out_avals),
            in_names=tuple(all_names),
            out_names=tuple(out_names),
            lowering_input_output_aliases=(),
            sim_require_finite=True,
            sim_require_nnan=True,
            nc=nc,
        )
        return tuple(outs)

    devices = jax.devices()[:N_CORES]
    mesh = Mesh(np.asarray(devices), ("core",))
    n_params, n_outs = len(in_names), len(out_avals)
    specs = (PartitionSpec("core"),) * (n_params + n_outs)
    out_specs = (PartitionSpec("core"),) * n_outs
    fn = jax.jit(
        shard_map(_body, mesh=mesh, in_specs=specs, out_specs=out_specs, check_rep=False),
        keep_unused=True,
    )
    return fn, in_names, out_names, out_avals, mesh


def kernel(pred_kpts, gt_kpts, kpt_mask, sigmas):
    pred = np.asarray(pred_kpts, dtype=np.float32)
    gt = np.asarray(gt_kpts, dtype=np.float32)
    mask = np.asarray(kpt_mask, dtype=np.float32)
    sig = np.asarray(sigmas, dtype=np.float32)
    if pred.shape != (N, K, 3) or not np.all(mask == 1.0):
        return _numpy_fallback(pred, gt, mask, sig)
    loss = _persistent_call(pred, gt, sig)
    return loss
